# revision 10
# baseline (speedup 1.0000x reference)
"""Trainium2 Bass kernel for nn_FFNPredictor (topk_masking).

Computes: p = x@w1 (two branches, rank 8) -> q = p@w2 -> g = silu(q0)*q1
          n2[i] = sum_tokens g^2 -> top-k channel indices (k = I*(1-sparsity)).

Sharding: token dim N=8192 split across 8 NeuronCores (1024 tokens each).
Each core computes a partial sum-of-squares vector [11008]; the host sums the
8 partials in float64 and does the (stable) top-k.

Precision strategy (ordering of top-k values must match an f32 reference):
  - p = x@w1 on the PE in bf16 hi/lo pairs: x = xh + xl (bf16 each, same
    total bytes as f32 so HBM traffic is unchanged), w1 = w1h + w1l.
    All four cross products accumulate in fp32 PSUM -> p accurate to ~2^-17.
  - q = p@w2 via fp32r (TF32-like: operands rounded to 11 explicit mantissa
    bits, exact multiply, fp32 accumulate).  Full fp32-grade precision is
    recovered by splitting both operands into 11-bit hi/lo halves
    (w2 on host, p on device via a Dekker split) and stacking the four
    cross-pairs along the contraction dim: K = 4*8 = 32, still one
    1-cycle/row matmul.
  - gate+square+reduce in ONE vector instruction via a custom DVE op:
    out = (silu(q0) * q1)^2, accum_out = running sum over tokens.
"""

import os
import sys

import numpy as np

if "/opt/trn_rl_repo" not in sys.path:
    sys.path.insert(0, "/opt/trn_rl_repo")

import ml_dtypes

# problem shapes (hardcoded per harness contract)
B, S, H, R, I = 4, 2048, 4096, 8, 11008
N = B * S               # 8192 tokens
NCORES = 8
NTOK = N // NCORES      # 1024 tokens per core
NCHUNK = H // 128       # 32 h-chunks
NIT = I // 128          # 86 i-tiles

TRACE = bool(int(os.environ.get("KERNEL_TRACE", "0")))
LAST_RESULT = None      # BassKernelResults of the most recent run (for test.py)

_NC = None              # cached finalized Bass object


def _register_ttsq():
    """Register the fused (in0*in1)^2 + sum custom DVE op (idempotent)."""
    from operator import add

    from concourse import dve_ops
    from concourse.dve_spec import C0, Spec, Src0, Src1, lower, sq
    from concourse.dve_uop import DveOpSpec

    name = "TT_SQ_REDUCE_ANT"
    for op in dve_ops.OPS:
        if op.name == name:
            return op

    def _ref(in0, in1, c0, c1, c2):
        b = ((in0.astype(np.float32) * in1).astype(np.float32) ** 2).astype(
            np.float32
        )
        return b, c0 + b.reshape(b.shape[0], -1).sum(axis=-1, keepdims=True)

    spec = Spec(body=sq(Src0 * Src1), accum=add, accum_init=C0, reference=_ref)
    shas = {
        ver: DveOpSpec(name=name, opcode=None, uops=lower(spec, ver=ver),
                       rd1_en=True).sha(ver)
        for ver in ("v3", "v4")
    }
    op = dve_ops.DveOp(name, spec, subdim=False, uops_sha=shas)
    dve_ops.OPS.append(op)
    dve_ops.CUSTOM_DVE_SPECS[name] = spec
    dve_ops._SUB_OPCODE_FOR_NAME[name] = (
        dve_ops._CUSTOM_DVE_ROW_BASE + len(dve_ops.OPS) - 1
    )
    assert max(dve_ops._SUB_OPCODE_FOR_NAME.values()) < 0x20
    return op


def _build_nc():
    import concourse.tile as tile
    from concourse import bacc, mybir

    ttsq = _register_ttsq()

    F32 = mybir.dt.float32
    F32R = mybir.dt.float32r
    BF16 = mybir.dt.bfloat16
    SILU = mybir.ActivationFunctionType.Silu

    nc = bacc.Bacc("TRN2", target_bir_lowering=False)

    d_xh = nc.dram_tensor("xh", [H, NTOK], BF16, kind="ExternalInput")
    d_xl = nc.dram_tensor("xl", [H, NTOK], BF16, kind="ExternalInput")
    d_w1 = nc.dram_tensor("w1s", [H, 64], BF16, kind="ExternalInput")
    d_w2_0 = nc.dram_tensor("w2s0", [32, I], F32R, kind="ExternalInput")
    d_w2_1 = nc.dram_tensor("w2s1", [32, I], F32R, kind="ExternalInput")
    d_acc = nc.dram_tensor("acc", [128, NIT], F32, kind="ExternalOutput")

    with tile.TileContext(nc) as tc:
        with (
            tc.tile_pool(name="weights", bufs=1) as wpool,
            tc.tile_pool(name="xin", bufs=4) as xpool,
            tc.tile_pool(name="pwork", bufs=1) as ppool,
            tc.tile_pool(name="bwork", bufs=3) as bpool,
            tc.tile_pool(name="psum_q", bufs=2, space="PSUM") as psq,
        ):
            # --- weights ---
            t_w1 = wpool.tile([128, NCHUNK, 64], BF16, tag="w1")
            nc.sync.dma_start(
                out=t_w1, in_=d_w1.rearrange("(c p) m -> p c m", p=128)
            )
            t_w2_0 = wpool.tile([32, I], F32R, tag="w20")
            nc.sync.dma_start(out=t_w2_0, in_=d_w2_0[:, :])
            t_w2_1 = wpool.tile([32, I], F32R, tag="w21")
            nc.sync.dma_start(out=t_w2_1, in_=d_w2_1[:, :])

            # --- phase A: p = x @ w1 over 32 h-chunks, xh+xl bf16 pairs ---
            # lhsT cols: 0-7 w1h[0], 8-15 w1h[1], 16-31 zero,
            #            32-39 w1l[0], 40-47 w1l[1], 48-63 zero.
            # psum rows 0-15: (xh+xl)@w1h, rows 32-47: (xh+xl)@w1l.
            # (engine APs need 32-aligned partition bases, hence M=64.)
            ps_p = psq.tile([64, NTOK], F32, tag="q0")
            for c in range(NCHUNK):
                th = xpool.tile([128, NTOK], BF16, tag="xh")
                tl = xpool.tile([128, NTOK], BF16, tag="xl")
                nc.sync.dma_start(out=th, in_=d_xh[c * 128:(c + 1) * 128, :])
                nc.sync.dma_start(out=tl, in_=d_xl[c * 128:(c + 1) * 128, :])
                for nh in range(2):
                    n0 = nh * (NTOK // 2)
                    n1 = n0 + NTOK // 2
                    nc.tensor.matmul(ps_p[:, n0:n1], t_w1[:, c, :],
                                     th[:, n0:n1],
                                     start=(c == 0), stop=False)
                    nc.tensor.matmul(ps_p[:, n0:n1], t_w1[:, c, :],
                                     tl[:, n0:n1],
                                     start=False, stop=(c == NCHUNK - 1))

            # p (rows 0-7 branch0, 8-15 branch1) = hi-part + lo-part
            p16 = ppool.tile([16, NTOK], F32, tag="p16")
            t16 = ppool.tile([16, NTOK], F32, tag="t16")
            s16 = ppool.tile([16, NTOK], F32, tag="s16")
            ph16 = ppool.tile([16, NTOK], F32, tag="ph16")
            pl16 = ppool.tile([16, NTOK], F32, tag="pl16")
            SUB = mybir.AluOpType.subtract
            ADD = mybir.AluOpType.add
            phi16 = ppool.tile([16, NTOK], F32, tag="phi16")
            nc.vector.tensor_copy(out=phi16, in_=ps_p[0:16, :])
            nc.vector.tensor_tensor(out=p16, in0=phi16,
                                    in1=ps_p[32:48, :], op=ADD)
            # Dekker split of p into 12+12 bit halves (fp32r-clean)
            nc.scalar.mul(out=t16, in_=p16, mul=4097.0)
            nc.vector.tensor_tensor(out=s16, in0=t16, in1=p16, op=SUB)
            nc.vector.tensor_tensor(out=ph16, in0=t16, in1=s16, op=SUB)
            nc.vector.tensor_tensor(out=pl16, in0=p16, in1=ph16, op=SUB)

            # build rhs stacks [ph_g; pl_g; ph_g; pl_g] via SBUF->SBUF DMA
            # (DMA has no partition-base alignment constraint)
            pstack0 = ppool.tile([32, NTOK], F32, tag="pstack0")
            pstack1 = ppool.tile([32, NTOK], F32, tag="pstack1")
            for g, pst in ((0, pstack0), (1, pstack1)):
                r = slice(g * 8, g * 8 + 8)
                nc.sync.dma_start(out=pst[0:8, :], in_=ph16[r, :])
                nc.sync.dma_start(out=pst[8:16, :], in_=pl16[r, :])
                nc.sync.dma_start(out=pst[16:24, :], in_=ph16[r, :])
                nc.sync.dma_start(out=pst[24:32, :], in_=pl16[r, :])

            pst_r0 = pstack0[:, :].bitcast(F32R)
            pst_r1 = pstack1[:, :].bitcast(F32R)

            # --- output accumulator ---
            t_acc = ppool.tile([128, NIT], F32, tag="acc")

            # --- phase B: per i-tile q0/q1 (K=32 stacked f32r), silu, fused
            #     multiply-square-reduce ---
            HALF = NTOK // 2
            for it in range(NIT):
                i0 = it * 128
                ps_q0 = psq.tile([128, NTOK], F32, tag="q0")
                ps_q1 = psq.tile([128, NTOK], F32, tag="q1")
                lhs0 = t_w2_0[:, i0:i0 + 128]
                lhs1 = t_w2_1[:, i0:i0 + 128]
                for hcol in range(2):
                    n0 = hcol * HALF
                    nc.tensor.matmul(ps_q0[:, n0:n0 + HALF], lhs0,
                                     pst_r0[:, n0:n0 + HALF],
                                     start=True, stop=True)
                    nc.tensor.matmul(ps_q1[:, n0:n0 + HALF], lhs1,
                                     pst_r1[:, n0:n0 + HALF],
                                     start=True, stop=True)
                t_sw = bpool.tile([128, NTOK], F32, tag="sw")
                nc.scalar.activation(out=t_sw, in_=ps_q0, func=SILU)
                t_body = bpool.tile([128, NTOK], F32, tag="body")
                nc.vector._custom_dve(
                    ttsq, out=t_body, in0=t_sw, in1=ps_q1,
                    s0=0.0, s1=0.0, imm2=0.0,
                    accum_out=t_acc[:, it:it + 1],
                )

            nc.sync.dma_start(out=d_acc[:, :], in_=t_acc)

    nc.finalize()
    return nc


def _get_nc():
    global _NC
    if _NC is None:
        _NC = _build_nc()
    return _NC


def _split11(v):
    """Split f32 array into 11-explicit-mantissa-bit hi + lo (fp32r-clean)."""
    v64 = np.asarray(v, np.float64)
    m, e = np.frexp(v64)
    m = np.round(m * (1 << 12)) / (1 << 12)
    h = np.ldexp(m, e).astype(np.float32)
    l = (v64 - h).astype(np.float32)
    return h, l


def kernel(x, w1, w2, sparsity_pct):
    global LAST_RESULT
    from concourse.bass_utils import run_bass_kernel_spmd

    x = np.asarray(x)
    w1 = np.asarray(w1, np.float32)
    w2 = np.asarray(w2, np.float32)
    sparsity = float(int(sparsity_pct)) / 100.0
    k = int(I * (1.0 - sparsity))

    xt = np.ascontiguousarray(x.reshape(N, H), dtype=np.float32)

    # x -> bf16 hi/lo pair, transposed to [H, N]
    xh = xt.astype(ml_dtypes.bfloat16)
    xl = (xt - xh.astype(np.float32)).astype(ml_dtypes.bfloat16)
    xh_t = np.ascontiguousarray(xh.T)
    xl_t = np.ascontiguousarray(xl.T)

    # w1 -> bf16 hi/lo, stacked with 32-aligned output bands:
    # cols 0-7 w1h[0], 8-15 w1h[1], 16-31 zero, 32-39 w1l[0], 40-47 w1l[1]
    w1h = w1.astype(ml_dtypes.bfloat16)
    w1l = (w1 - w1h.astype(np.float32)).astype(ml_dtypes.bfloat16)
    w1s = np.zeros((H, 64), dtype=ml_dtypes.bfloat16)
    w1s[:, 0:8] = w1h[0]
    w1s[:, 8:16] = w1h[1]
    w1s[:, 32:40] = w1l[0]
    w1s[:, 40:48] = w1l[1]

    # w2 -> 11-bit hi/lo, stacked rows [w2h; w2h; w2l; w2l] per branch
    w2s = []
    for g in range(2):
        h, l = _split11(w2[g])
        s = np.empty((32, I), np.float32)
        s[0:8] = h
        s[8:16] = h
        s[16:24] = l
        s[24:32] = l
        w2s.append(s)

    nc = _get_nc()
    in_maps = []
    for c in range(NCORES):
        sl = slice(c * NTOK, (c + 1) * NTOK)
        in_maps.append({
            "xh": np.ascontiguousarray(xh_t[:, sl]),
            "xl": np.ascontiguousarray(xl_t[:, sl]),
            "w1s": w1s,
            "w2s0": w2s[0],
            "w2s1": w2s[1],
        })

    res = run_bass_kernel_spmd(nc, in_maps, core_ids=list(range(NCORES)),
                               trace=TRACE)
    LAST_RESULT = res

    n2 = np.zeros(I, np.float64)
    for c in range(NCORES):
        acc = res.results[c]["acc"]          # [128, NIT]
        n2 += acc.astype(np.float64).T.reshape(-1)

    idx = np.argsort(-n2, kind="stable")[:k].astype(np.int32)
    return idx
